# revision 48
# baseline (speedup 1.0000x reference)
"""Causal attention (B=4, Sq=Sk=2048, D=1024, f32) on 8 TRN2 NeuronCores.

Strategy: pure data-parallel (no collectives). Each core handles one
(batch, half) shard: batch b = core//2, and half of the query rows of
that batch, chosen as an interleaving of 128-row tiles that balances
the causal workload. All 8 cores run the same program (SPMD); per-core
variation (which query rows, causal mask offsets) is carried entirely
in the data.

Per-core schedule: 8 query tiles of 128 rows, slot s covering keys
[0, 256*(s+1)).  A core's 8 query tiles are assigned to slots so that
each tile's causal need (gq+128 keys) fits its slot.  The causal
boundary is applied with an additive -1e9 mask (host-computed per slot)
on the final key tile of each slot.

Compute: S = Q K^T via bf16 matmuls on host-pre-transposed Q/K layouts
(host casts Q/K/V to bf16 — halves the HBM stream, which otherwise
binds at the ~360 GB/s per-core limit; measured output error 3.1e-3);
softmax without max-subtraction (logits S/32 ~ N(0,1), exp is safe)
with the row-sum fused into the exp activation (accum_out); P^T via
TensorE transpose (keeps the PE stream dense so the HAM clock gate
stays at 2.4 GHz — DMA-transpose latency starved the PE in v1); P^T V
accumulated per-stage in PSUM then into an SBUF accumulator (slots
interleave, so PSUM can't hold all 8); final 1/rowsum folded in with
one fused multiply-add.  The (S, exp, transpose, PV) chain is
software-pipelined two stages deep so the PE never waits on ACT/DVE.
All DMA uses few, large, fully-contiguous transfers on the two HWDGE
rings, ordered to match the stage schedule's need order.
"""

import os
import numpy as np
import ml_dtypes

B, SQ, SK, D = 4, 2048, 2048, 1024
NCORES = 8
P = 128                      # partitions / tile rows
NDC = D // P                 # 8 d-chunks of 128
NKC = SK // P                # 16 k-chunks of 128
KTILE = 512                  # key tile (free dim of S matmul)
NSLOT = 8                    # query tiles per core
SLOT_KLEN = [256 * (s + 1) for s in range(NSLOT)]   # keys covered per slot
# query-tile (128-row) indices of the batch handled by core parity j,
# ordered by slot (ascending causal need); complement pairs sum equally.
TILES_J0 = [0, 3, 5, 6, 8, 11, 13, 14]
TILES_J1 = [1, 2, 4, 7, 9, 10, 12, 15]
NEG = -1.0e9
SCALE = 1.0 / 32.0           # 1/sqrt(D)

_CACHE = {}


def _build_nc():
    import concourse.bacc as bacc
    import concourse.tile as tile
    import concourse.mybir as mybir
    from concourse.masks import make_identity

    dt = mybir.dt
    nc = bacc.Bacc("TRN2", target_bir_lowering=False, debug=False,
                   num_devices=NCORES)

    qt_ext = nc.dram_tensor("qt", [NSLOT, P, NDC, P], dt.bfloat16,
                            kind="ExternalInput").ap()
    kt_ext = nc.dram_tensor("kt", [SK // KTILE, P, NDC, KTILE], dt.bfloat16,
                            kind="ExternalInput").ap()
    v_ext = nc.dram_tensor("v", [SK // KTILE, P, NKC // 4, D], dt.bfloat16,
                           kind="ExternalInput").ap()
    thr_ext = nc.dram_tensor("thr", [P, NSLOT], dt.float32,
                             kind="ExternalInput").ap()
    out_ext = nc.dram_tensor("out", [NSLOT * P, D], dt.float32,
                             kind="ExternalOutput").ap()

    # stage = (slot, k-tile index, k0, kw, last); sorted by key-prefix
    # need so big slots interleave with small ones — keeps instantaneous
    # DMA demand behind compute while the kt/v prefixes stream in.
    stages = []
    for s in range(NSLOT):
        klen = SLOT_KLEN[s]
        nk = (klen + KTILE - 1) // KTILE
        for kt in range(nk):
            k0 = kt * KTILE
            kw = min(KTILE, klen - k0)
            stages.append((s, kt, k0, kw, kt == nk - 1))
    stages.sort(key=lambda st: (st[2] + st[3], st[0]))
    NK = {}
    for s, kt, k0, kw, last in stages:
        NK[s] = max(NK.get(s, 0), kt + 1)
    # fused-finish safety: each slot's second-to-last stage must retire its
    # o_acc add (emitted at stage idx+2) strictly before the last stage's
    # prescale (emitted in emit_t at stage idx+1, which the emission loop
    # places before emit_pv of the same iteration)
    for s in range(NSLOT):
        idx = [i for i, st in enumerate(stages) if st[0] == s]
        assert len(idx) == 1 or idx[-2] + 1 < idx[-1], (s, idx)

    with tile.TileContext(nc) as tc:
        with tc.tile_pool(name="big", bufs=1) as big, \
             tc.tile_pool(name="work", bufs=4) as work, \
             tc.tile_pool(name="acc", bufs=2) as acc, \
             tc.tile_pool(name="spsum", bufs=2, space="PSUM") as spsum, \
             tc.tile_pool(name="tpsum", bufs=2, space="PSUM") as tpsum, \
             tc.tile_pool(name="opsum", bufs=2, space="PSUM") as opsum:

            qt_sb = big.tile([P, NSLOT, NDC, P], dt.bfloat16)
            kt_sb = big.tile([P, SK // KTILE, NDC, KTILE], dt.bfloat16)
            v_sb = big.tile([P, SK // KTILE, 4, D], dt.bfloat16)
            mask_sb = big.tile([P, NSLOT, 256], dt.bfloat16)
            thr_sb = big.tile([P, NSLOT], dt.float32)
            m0i = big.tile([P, 256], dt.int32)
            m0f = big.tile([P, 256], dt.float32)
            rsums = big.tile([P, NSLOT, 4], dt.float32)
            o_acc = big.tile([P, NSLOT, D], dt.float32)
            ident = big.tile([P, P], dt.bfloat16)
            make_identity(nc, ident[:])
            # on-device causal masks: mask[p, s, f] = -1e9 where
            # (f - p) > thr[s], thr[s] = gq_s - klen_s + 256 (per-core data).
            # Generated during the DMA head on otherwise-idle engines.
            nc.gpsimd.iota(m0i[:], pattern=[[1, 256]], base=0,
                           channel_multiplier=-1)
            nc.vector.tensor_copy(m0f[:], m0i[:])

            # All loads upfront as big contiguous transfers. The two HWDGE
            # rings progress together (SDMA round-robins between rings at
            # packet granularity), so interleave the two queues by GLOBAL
            # need rank — a low-priority load on one ring steals bandwidth
            # from a critical one on the other.
            def qt_pair(i):
                return (qt_sb[:, 2 * i:2 * i + 2],
                        qt_ext[2 * i:2 * i + 2].rearrange("s p c q -> p s c q"))

            nc.scalar.dma_start(thr_sb[:], thr_ext)
            nc.sync.dma_start(kt_sb[:, 0], kt_ext[0])
            nc.scalar.dma_start(*qt_pair(0))
            nc.scalar.dma_start(v_sb[:, 0], v_ext[0])
            nc.sync.dma_start(*qt_pair(1))
            nc.scalar.dma_start(*qt_pair(2))
            nc.sync.dma_start(kt_sb[:, 1], kt_ext[1])
            nc.sync.dma_start(*qt_pair(3))
            nc.sync.dma_start(v_sb[:, 1], v_ext[1])
            nc.sync.dma_start(kt_sb[:, 2], kt_ext[2])
            nc.sync.dma_start(v_sb[:, 2], v_ext[2])
            nc.sync.dma_start(kt_sb[:, 3], kt_ext[3])
            nc.sync.dma_start(v_sb[:, 3], v_ext[3])
            # NOTE: the scalar (ACT) engine must issue no further DMAs —
            # a blocked DMA issue in its FIFO would stall the exps behind it

            # Pre-warm the PE HAM clock gate: ~5.5 us of identity matmuls
            # that finish just before the first real data arrives (the idle
            # gap to the first S matmul stays under the ~3.4 us re-throttle
            # window, so the real matmuls start at 2.4 GHz).
            warm_ps = spsum.tile([P, KTILE], dt.float32, tag="s")
            scratch = big.tile([P, KTILE], dt.bfloat16)
            nc.vector.memset(scratch[:, 0:1], 0.0)
            for w in range(10):
                nc.tensor.matmul(warm_ps[:], ident[:], scratch[:],
                                 start=True, stop=True)

            state = {}               # per-stage-index carried tiles

            def emit_s(i):
                s, kt, k0, kw, last = stages[i]
                s_ps = spsum.tile([P, KTILE], dt.float32, tag="s")
                blk, off = divmod(k0, KTILE)
                for c in range(NDC):
                    nc.tensor.matmul(s_ps[:, :kw],
                                     qt_sb[:, s, c],
                                     kt_sb[:, blk, c, off:off + kw],
                                     start=(c == 0), stop=(c == NDC - 1))
                if last:
                    # build this slot's causal mask just-in-time (keeps the
                    # DVE free early; mask depends only on thr + iota)
                    nc.vector.tensor_scalar(mask_sb[:, s], m0f[:],
                                            thr_sb[:, s:s + 1], NEG,
                                            op0=mybir.AluOpType.is_gt,
                                            op1=mybir.AluOpType.mult)
                    nc.vector.tensor_tensor(s_ps[:, kw - 256:kw],
                                            s_ps[:, kw - 256:kw],
                                            mask_sb[:, s],
                                            op=mybir.AluOpType.add)
                p_t = work.tile([P, KTILE], dt.bfloat16, tag="p")
                nc.scalar.activation(p_t[:, :kw], s_ps[:, :kw],
                                     mybir.ActivationFunctionType.Exp,
                                     scale=SCALE,
                                     accum_out=rsums[:, s, kt:kt + 1])
                state[("p", i)] = p_t

            def emit_t(i):
                s, kt, k0, kw, last = stages[i]
                p_t = state.pop(("p", i))
                nch = kw // P
                pt_ps = tpsum.tile([P, KTILE // P, P], dt.bfloat16, tag="tp")
                for c in range(nch):
                    nc.tensor.transpose(pt_ps[:, c], p_t[:, c * P:(c + 1) * P],
                                        ident[:])
                pt_t = work.tile([P, KTILE // P, P], dt.bfloat16, tag="pt")
                nc.vector.tensor_copy(pt_t[:, :nch], pt_ps[:, :nch])
                state[("pt", i)] = pt_t
                if last:
                    # the slot's rowsum is complete (its exp just ran);
                    # fold 1/rowsum into o_acc off the critical path so the
                    # last PV only needs one fused multiply-add
                    nk = NK[s]
                    emit_recip(s)
                    if nk > 1:
                        nc.vector.tensor_scalar(o_acc[:, s], o_acc[:, s],
                                                recips[s][:], None,
                                                op0=mybir.AluOpType.mult)

            recips = {}

            def emit_recip(s):
                nk = NK[s]
                recip = work.tile([P, 1], dt.float32, name=f"recip{s}",
                                  tag="recip")
                if nk == 1:
                    nc.vector.reciprocal(recip[:], rsums[:, s, :1])
                else:
                    rtot = work.tile([P, 1], dt.float32, tag="rtot")
                    nc.vector.tensor_reduce(rtot[:], rsums[:, s, :nk],
                                            axis=mybir.AxisListType.X,
                                            op=mybir.AluOpType.add)
                    nc.vector.reciprocal(recip[:], rtot[:])
                recips[s] = recip

            def emit_pv(i):
                s, kt, k0, kw, last = stages[i]
                nk = NK[s]
                o_ps = opsum.tile([P, D], dt.float32, tag="o")
                pt_t = state.pop(("pt", i))
                nch = kw // P
                for c in range(nch):
                    kc = k0 // P + c
                    for h in range(2):
                        nc.tensor.matmul(
                            o_ps[:, h * KTILE:(h + 1) * KTILE],
                            pt_t[:, c],
                            v_sb[:, kc // 4, kc % 4,
                                 h * KTILE:(h + 1) * KTILE],
                            start=(c == 0), stop=(c == nch - 1))
                if last:
                    o_sb = acc.tile([P, D], dt.float32, tag="o_sb")
                    if nk == 1:
                        emit_recip(s)
                    # finish in d-halves so the first 256 KB store starts
                    # while the second half is still normalizing
                    for h in range(2):
                        hs = slice(h * KTILE, (h + 1) * KTILE)
                        if nk == 1:
                            nc.vector.tensor_scalar(o_sb[:, hs], o_ps[:, hs],
                                                    recips[s][:], None,
                                                    op0=mybir.AluOpType.mult)
                        else:
                            # o_acc was pre-scaled by 1/rowsum when the
                            # second-to-last stage retired
                            nc.vector.scalar_tensor_tensor(
                                o_sb[:, hs], o_ps[:, hs], recips[s][:],
                                o_acc[:, s, hs],
                                op0=mybir.AluOpType.mult,
                                op1=mybir.AluOpType.add)
                        nc.sync.dma_start(
                            out_ext[s * P:(s + 1) * P, hs], o_sb[:, hs])
                    return
                if kt == 0:
                    nc.vector.tensor_copy(o_acc[:, s], o_ps[:])
                else:
                    nc.vector.tensor_tensor(o_acc[:, s], o_acc[:, s], o_ps[:],
                                            op=mybir.AluOpType.add)


            n = len(stages)
            for i in range(n + 2):
                if i < n:
                    emit_s(i)
                if 1 <= i <= n:
                    emit_t(i - 1)
                if i >= 2:
                    emit_pv(i - 2)

    nc.compile()
    return nc


def _install_axon_hooks_shim():
    """concourse's trace path imports antenv.axon_hooks, which this image
    lacks; provide it (backed by the libaxon ctypes hook when available)
    so run_bass_kernel_spmd(trace=True) degrades gracefully."""
    import sys, types
    if "antenv.axon_hooks" in sys.modules:
        return
    hook = None
    try:
        from trn_agent_boot.trn_boot import _ntff_profile_via_ctypes
        hook = _ntff_profile_via_ctypes("/opt/axon/libaxon_pjrt.so")
    except Exception:
        hook = None
    mod = types.ModuleType("antenv.axon_hooks")
    mod.get_axon_ntff_profile_hook = lambda: hook
    mod.set_axon_ntff_profile_hook = lambda h: None
    sys.modules["antenv.axon_hooks"] = mod


def _get_nc():
    if "nc" not in _CACHE:
        os.environ.setdefault("JAX_COMPILATION_CACHE_DIR", "/tmp/jax_comp_cache")
        try:
            import jax
            jax.config.update("jax_compilation_cache_dir", "/tmp/jax_comp_cache")
            jax.config.update("jax_persistent_cache_min_entry_size_bytes", -1)
            jax.config.update("jax_persistent_cache_min_compile_time_secs", 0)
        except Exception:
            pass
        _install_axon_hooks_shim()
        _CACHE["nc"] = _build_nc()
    return _CACHE["nc"]


def _host_thr(tiles):
    """[128, NSLOT] per-slot causal thresholds: mask where (f-p) > thr[s]."""
    thr = np.empty((P, NSLOT), np.float32)
    for s in range(NSLOT):
        thr[:, s] = P * tiles[s] - SLOT_KLEN[s] + 256
    return thr


def make_in_maps(query, key, value):
    query = np.asarray(query, np.float32)
    key = np.asarray(key, np.float32)
    value = np.asarray(value, np.float32)
    in_maps = []
    for core in range(NCORES):
        b, j = divmod(core, 2)
        tiles = TILES_J0 if j == 0 else TILES_J1
        qrows = np.concatenate([query[b, P * t:P * (t + 1)] for t in tiles])
        # qt[s, p, c, q] = qrows[s*128+q, c*128+p]
        qt = np.ascontiguousarray(
            qrows.astype(ml_dtypes.bfloat16)
            .reshape(NSLOT, P, NDC, P).transpose(0, 3, 2, 1))
        # kt[blk, p, c, k] = key[b, blk*512+k, c*128+p]
        kt = np.ascontiguousarray(
            key[b].astype(ml_dtypes.bfloat16)
            .reshape(SK // KTILE, KTILE, NDC, P).transpose(0, 3, 2, 1))
        # v[blk, p, kc, d] = value[b, blk*512 + kc*128 + p, d]
        v = np.ascontiguousarray(
            value[b].astype(ml_dtypes.bfloat16)
            .reshape(SK // KTILE, 4, P, D).transpose(0, 2, 1, 3))
        in_maps.append({
            "qt": qt,
            "kt": kt,
            "v": v,
            "thr": _host_thr(tiles),
        })
    return in_maps


def assemble(results):
    out = np.empty((B, SQ, D), np.float32)
    for core in range(NCORES):
        b, j = divmod(core, 2)
        tiles = TILES_J0 if j == 0 else TILES_J1
        o = results[core]["out"]
        for s, t in enumerate(tiles):
            out[b, P * t:P * (t + 1)] = o[P * s:P * (s + 1)]
    return out


def _get_runner(nc):
    """Build once: a jitted SPMD executable over the 8 axon devices
    (mirrors bass2jax.run_bass_via_pjrt, but cached across kernel() calls
    so repeat calls skip tracing/compilation)."""
    if "runner" in _CACHE:
        return _CACHE["runner"]
    import jax
    import concourse.mybir as mybir
    from concourse import bass2jax
    from jax.sharding import Mesh, PartitionSpec
    from jax.experimental.shard_map import shard_map
    import numpy as _np

    bass2jax.install_neuronx_cc_hook()
    partition_name = (nc.partition_id_tensor.name
                      if nc.partition_id_tensor else None)
    in_names, out_names, out_avals, zero_outs = [], [], [], []
    for alloc in nc.m.functions[0].allocations:
        if not isinstance(alloc, mybir.MemoryLocationSet):
            continue
        name = alloc.memorylocations[0].name
        if alloc.kind == "ExternalInput":
            if name != partition_name:
                in_names.append(name)
        elif alloc.kind == "ExternalOutput":
            out_names.append(name)
            shape = tuple(alloc.tensor_shape)
            dtype = mybir.dt.np(alloc.dtype)
            out_avals.append(jax.core.ShapedArray(shape, dtype))
            zero_outs.append(_np.zeros(shape, dtype))
    n_params = len(in_names)
    all_names = in_names + out_names
    if partition_name is not None:
        all_names = all_names + [partition_name]

    def _body(*args):
        operands = list(args)
        if partition_name is not None:
            operands.append(bass2jax.partition_id_tensor())
        outs = bass2jax._bass_exec_p.bind(
            *operands,
            out_avals=tuple(out_avals),
            in_names=tuple(all_names),
            out_names=tuple(out_names),
            lowering_input_output_aliases=(),
            sim_require_finite=True,
            sim_require_nnan=True,
            nc=nc,
        )
        return tuple(outs)

    devices = jax.devices()[:NCORES]
    mesh = Mesh(_np.asarray(devices), ("core",))
    n_outs = len(out_names)
    sharded = jax.jit(
        shard_map(_body, mesh=mesh,
                  in_specs=(PartitionSpec("core"),) * (n_params + n_outs),
                  out_specs=(PartitionSpec("core"),) * n_outs,
                  check_rep=False),
        donate_argnums=tuple(range(n_params, n_params + n_outs)),
        keep_unused=True,
    )
    _CACHE["runner"] = (sharded, in_names, out_names, out_avals, zero_outs)
    return _CACHE["runner"]


def kernel(query, key, value, _run_kwargs=None):
    import numpy as _np
    nc = _get_nc()
    in_maps = make_in_maps(query, key, value)
    if _run_kwargs is not None:
        # profiling path for test.py
        from concourse.bass_utils import run_bass_kernel_spmd
        res = run_bass_kernel_spmd(nc, in_maps, list(range(NCORES)),
                                   **dict(_run_kwargs))
        _CACHE["last_result"] = res
        return assemble(res.results)
    sharded, in_names, out_names, out_avals, zero_outs = _get_runner(nc)
    concat_in = [
        _np.concatenate([m[name] for m in in_maps], axis=0)
        for name in in_names
    ]
    concat_zeros = [
        _np.zeros((NCORES * z.shape[0], *z.shape[1:]), z.dtype)
        for z in zero_outs
    ]
    out_arrs = sharded(*concat_in, *concat_zeros)
    results = [
        {name: _np.asarray(out_arrs[i]).reshape(NCORES, *out_avals[i].shape)[c]
         for i, name in enumerate(out_names)}
        for c in range(NCORES)
    ]
    return assemble(results)


# revision 49
# speedup vs baseline: 1.0256x; 1.0256x over previous
"""Causal attention (B=4, Sq=Sk=2048, D=1024, f32) on 8 TRN2 NeuronCores.

Strategy: pure data-parallel (no collectives). Each core handles one
(batch, half) shard: batch b = core//2, and half of the query rows of
that batch, chosen as an interleaving of 128-row tiles that balances
the causal workload. All 8 cores run the same program (SPMD); per-core
variation (which query rows, causal mask offsets) is carried entirely
in the data.

Per-core schedule: 8 query tiles of 128 rows, slot s covering keys
[0, 256*(s+1)).  A core's 8 query tiles are assigned to slots so that
each tile's causal need (gq+128 keys) fits its slot.  The causal
boundary is applied with an additive -1e9 mask (host-computed per slot)
on the final key tile of each slot.

Compute: S = Q K^T via bf16 matmuls on host-pre-transposed Q/K layouts
(host casts Q/K/V to bf16 — halves the HBM stream, which otherwise
binds at the ~360 GB/s per-core limit; measured output error 3.1e-3);
softmax without max-subtraction (logits S/32 ~ N(0,1), exp is safe)
with the row-sum fused into the exp activation (accum_out); P^T via
TensorE transpose (keeps the PE stream dense so the HAM clock gate
stays at 2.4 GHz — DMA-transpose latency starved the PE in v1); P^T V
accumulated per-stage in PSUM then into an SBUF accumulator (slots
interleave, so PSUM can't hold all 8); final 1/rowsum folded in with
one fused multiply-add.  The (S, exp, transpose, PV) chain is
software-pipelined two stages deep so the PE never waits on ACT/DVE.
All DMA uses few, large, fully-contiguous transfers on the two HWDGE
rings, ordered to match the stage schedule's need order.
"""

import os
import numpy as np
import ml_dtypes

B, SQ, SK, D = 4, 2048, 2048, 1024
NCORES = 8
P = 128                      # partitions / tile rows
NDC = D // P                 # 8 d-chunks of 128
NKC = SK // P                # 16 k-chunks of 128
KTILE = 512                  # key tile (free dim of S matmul)
NSLOT = 8                    # query tiles per core
SLOT_KLEN = [256 * (s + 1) for s in range(NSLOT)]   # keys covered per slot
# query-tile (128-row) indices of the batch handled by core parity j,
# ordered by slot (ascending causal need); complement pairs sum equally.
TILES_J0 = [0, 3, 5, 6, 8, 11, 13, 14]
TILES_J1 = [1, 2, 4, 7, 9, 10, 12, 15]
NEG = -1.0e9
SCALE = 1.0 / 32.0           # 1/sqrt(D)

_CACHE = {}


def _build_nc():
    import concourse.bacc as bacc
    import concourse.tile as tile
    import concourse.mybir as mybir
    from concourse.masks import make_identity

    dt = mybir.dt
    nc = bacc.Bacc("TRN2", target_bir_lowering=False, debug=False,
                   num_devices=NCORES)

    qt_ext = nc.dram_tensor("qt", [NSLOT, P, NDC, P], dt.bfloat16,
                            kind="ExternalInput").ap()
    kt_ext = nc.dram_tensor("kt", [SK // KTILE, P, NDC, KTILE], dt.bfloat16,
                            kind="ExternalInput").ap()
    v_ext = nc.dram_tensor("v", [SK // KTILE, P, NKC // 4, D], dt.bfloat16,
                           kind="ExternalInput").ap()
    thr_ext = nc.dram_tensor("thr", [P, NSLOT], dt.float32,
                             kind="ExternalInput").ap()
    out_ext = nc.dram_tensor("out", [NSLOT * P, D], dt.float32,
                             kind="ExternalOutput").ap()

    # stage = (slot, k-tile index, k0, kw, last); sorted by key-prefix
    # need so big slots interleave with small ones — keeps instantaneous
    # DMA demand behind compute while the kt/v prefixes stream in.
    stages = []
    for s in range(NSLOT):
        klen = SLOT_KLEN[s]
        nk = (klen + KTILE - 1) // KTILE
        for kt in range(nk):
            k0 = kt * KTILE
            kw = min(KTILE, klen - k0)
            stages.append((s, kt, k0, kw, kt == nk - 1))
    stages.sort(key=lambda st: (st[2] + st[3], st[0]))
    NK = {}
    for s, kt, k0, kw, last in stages:
        NK[s] = max(NK.get(s, 0), kt + 1)
    # fused-finish safety: each slot's second-to-last stage must retire its
    # o_acc add (emitted at stage idx+2) strictly before the last stage's
    # prescale (emitted in emit_t at stage idx+1, which the emission loop
    # places before emit_pv of the same iteration)
    for s in range(NSLOT):
        idx = [i for i, st in enumerate(stages) if st[0] == s]
        assert len(idx) == 1 or idx[-2] + 1 < idx[-1], (s, idx)

    with tile.TileContext(nc) as tc:
        with tc.tile_pool(name="big", bufs=1) as big, \
             tc.tile_pool(name="work", bufs=4) as work, \
             tc.tile_pool(name="acc", bufs=2) as acc, \
             tc.tile_pool(name="spsum", bufs=2, space="PSUM") as spsum, \
             tc.tile_pool(name="tpsum", bufs=2, space="PSUM") as tpsum, \
             tc.tile_pool(name="opsum", bufs=2, space="PSUM") as opsum:

            qt_sb = big.tile([P, NSLOT, NDC, P], dt.bfloat16)
            kt_sb = big.tile([P, SK // KTILE, NDC, KTILE], dt.bfloat16)
            v_sb = big.tile([P, SK // KTILE, 4, D], dt.bfloat16)
            mask_sb = big.tile([P, NSLOT, 256], dt.bfloat16)
            thr_sb = big.tile([P, NSLOT], dt.float32)
            m0i = big.tile([P, 256], dt.int32)
            m0f = big.tile([P, 256], dt.float32)
            rsums = big.tile([P, NSLOT, 4], dt.float32)
            o_acc = big.tile([P, NSLOT, D], dt.float32)
            ident = big.tile([P, P], dt.bfloat16)
            make_identity(nc, ident[:])
            # on-device causal masks: mask[p, s, f] = -1e9 where
            # (f - p) > thr[s], thr[s] = gq_s - klen_s + 256 (per-core data).
            # Generated during the DMA head on otherwise-idle engines.
            nc.gpsimd.iota(m0i[:], pattern=[[1, 256]], base=0,
                           channel_multiplier=-1)
            nc.vector.tensor_copy(m0f[:], m0i[:])

            # All loads upfront as big contiguous transfers. The two HWDGE
            # rings progress together (SDMA round-robins between rings at
            # packet granularity), so interleave the two queues by GLOBAL
            # need rank — a low-priority load on one ring steals bandwidth
            # from a critical one on the other.
            def qt_pair(i):
                return (qt_sb[:, 2 * i:2 * i + 2],
                        qt_ext[2 * i:2 * i + 2].rearrange("s p c q -> p s c q"))

            nc.sync.dma_start(kt_sb[:, 0], kt_ext[0])
            nc.scalar.dma_start(*qt_pair(0))
            nc.scalar.dma_start(thr_sb[:], thr_ext)
            nc.scalar.dma_start(v_sb[:, 0], v_ext[0])
            nc.sync.dma_start(*qt_pair(1))
            nc.scalar.dma_start(*qt_pair(2))
            nc.sync.dma_start(kt_sb[:, 1], kt_ext[1])
            nc.sync.dma_start(*qt_pair(3))
            nc.sync.dma_start(v_sb[:, 1], v_ext[1])
            nc.sync.dma_start(kt_sb[:, 2], kt_ext[2])
            nc.sync.dma_start(v_sb[:, 2], v_ext[2])
            nc.sync.dma_start(kt_sb[:, 3], kt_ext[3])
            nc.sync.dma_start(v_sb[:, 3], v_ext[3])
            # NOTE: the scalar (ACT) engine must issue no further DMAs —
            # a blocked DMA issue in its FIFO would stall the exps behind it

            # Pre-warm the PE HAM clock gate: ~5.5 us of identity matmuls
            # that finish just before the first real data arrives (the idle
            # gap to the first S matmul stays under the ~3.4 us re-throttle
            # window, so the real matmuls start at 2.4 GHz).
            warm_ps = spsum.tile([P, KTILE], dt.float32, tag="s")
            scratch = big.tile([P, KTILE], dt.bfloat16)
            nc.vector.memset(scratch[:, 0:1], 0.0)
            for w in range(10):
                nc.tensor.matmul(warm_ps[:], ident[:], scratch[:],
                                 start=True, stop=True)

            state = {}               # per-stage-index carried tiles

            def emit_s(i):
                s, kt, k0, kw, last = stages[i]
                s_ps = spsum.tile([P, KTILE], dt.float32, tag="s")
                blk, off = divmod(k0, KTILE)
                for c in range(NDC):
                    nc.tensor.matmul(s_ps[:, :kw],
                                     qt_sb[:, s, c],
                                     kt_sb[:, blk, c, off:off + kw],
                                     start=(c == 0), stop=(c == NDC - 1))
                if last:
                    # build this slot's causal mask just-in-time (keeps the
                    # DVE free early; mask depends only on thr + iota)
                    nc.vector.tensor_scalar(mask_sb[:, s], m0f[:],
                                            thr_sb[:, s:s + 1], NEG,
                                            op0=mybir.AluOpType.is_gt,
                                            op1=mybir.AluOpType.mult)
                    nc.vector.tensor_tensor(s_ps[:, kw - 256:kw],
                                            s_ps[:, kw - 256:kw],
                                            mask_sb[:, s],
                                            op=mybir.AluOpType.add)
                p_t = work.tile([P, KTILE], dt.bfloat16, tag="p")
                nc.scalar.activation(p_t[:, :kw], s_ps[:, :kw],
                                     mybir.ActivationFunctionType.Exp,
                                     scale=SCALE,
                                     accum_out=rsums[:, s, kt:kt + 1])
                state[("p", i)] = p_t

            def emit_t(i):
                s, kt, k0, kw, last = stages[i]
                p_t = state.pop(("p", i))
                nch = kw // P
                pt_ps = tpsum.tile([P, KTILE // P, P], dt.bfloat16, tag="tp")
                for c in range(nch):
                    nc.tensor.transpose(pt_ps[:, c], p_t[:, c * P:(c + 1) * P],
                                        ident[:])
                pt_t = work.tile([P, KTILE // P, P], dt.bfloat16, tag="pt")
                nc.vector.tensor_copy(pt_t[:, :nch], pt_ps[:, :nch])
                state[("pt", i)] = pt_t
                if last:
                    # the slot's rowsum is complete (its exp just ran);
                    # fold 1/rowsum into o_acc off the critical path so the
                    # last PV only needs one fused multiply-add
                    nk = NK[s]
                    emit_recip(s)
                    if nk > 1:
                        nc.vector.tensor_scalar(o_acc[:, s], o_acc[:, s],
                                                recips[s][:], None,
                                                op0=mybir.AluOpType.mult)

            recips = {}

            def emit_recip(s):
                nk = NK[s]
                recip = work.tile([P, 1], dt.float32, name=f"recip{s}",
                                  tag="recip")
                if nk == 1:
                    nc.vector.reciprocal(recip[:], rsums[:, s, :1])
                else:
                    rtot = work.tile([P, 1], dt.float32, tag="rtot")
                    nc.vector.tensor_reduce(rtot[:], rsums[:, s, :nk],
                                            axis=mybir.AxisListType.X,
                                            op=mybir.AluOpType.add)
                    nc.vector.reciprocal(recip[:], rtot[:])
                recips[s] = recip

            def emit_pv(i):
                s, kt, k0, kw, last = stages[i]
                nk = NK[s]
                o_ps = opsum.tile([P, D], dt.float32, tag="o")
                pt_t = state.pop(("pt", i))
                nch = kw // P
                for c in range(nch):
                    kc = k0 // P + c
                    for h in range(2):
                        nc.tensor.matmul(
                            o_ps[:, h * KTILE:(h + 1) * KTILE],
                            pt_t[:, c],
                            v_sb[:, kc // 4, kc % 4,
                                 h * KTILE:(h + 1) * KTILE],
                            start=(c == 0), stop=(c == nch - 1))
                if last:
                    o_sb = acc.tile([P, D], dt.float32, tag="o_sb")
                    if nk == 1:
                        emit_recip(s)
                    # finish in d-halves so the first 256 KB store starts
                    # while the second half is still normalizing
                    for h in range(2):
                        hs = slice(h * KTILE, (h + 1) * KTILE)
                        if nk == 1:
                            nc.vector.tensor_scalar(o_sb[:, hs], o_ps[:, hs],
                                                    recips[s][:], None,
                                                    op0=mybir.AluOpType.mult)
                        else:
                            # o_acc was pre-scaled by 1/rowsum when the
                            # second-to-last stage retired
                            nc.vector.scalar_tensor_tensor(
                                o_sb[:, hs], o_ps[:, hs], recips[s][:],
                                o_acc[:, s, hs],
                                op0=mybir.AluOpType.mult,
                                op1=mybir.AluOpType.add)
                        nc.sync.dma_start(
                            out_ext[s * P:(s + 1) * P, hs], o_sb[:, hs])
                    return
                if kt == 0:
                    nc.vector.tensor_copy(o_acc[:, s], o_ps[:])
                else:
                    nc.vector.tensor_tensor(o_acc[:, s], o_acc[:, s], o_ps[:],
                                            op=mybir.AluOpType.add)


            n = len(stages)
            for i in range(n + 2):
                if i < n:
                    emit_s(i)
                if 1 <= i <= n:
                    emit_t(i - 1)
                if i >= 2:
                    emit_pv(i - 2)

    nc.compile()
    return nc


def _install_axon_hooks_shim():
    """concourse's trace path imports antenv.axon_hooks, which this image
    lacks; provide it (backed by the libaxon ctypes hook when available)
    so run_bass_kernel_spmd(trace=True) degrades gracefully."""
    import sys, types
    if "antenv.axon_hooks" in sys.modules:
        return
    hook = None
    try:
        from trn_agent_boot.trn_boot import _ntff_profile_via_ctypes
        hook = _ntff_profile_via_ctypes("/opt/axon/libaxon_pjrt.so")
    except Exception:
        hook = None
    mod = types.ModuleType("antenv.axon_hooks")
    mod.get_axon_ntff_profile_hook = lambda: hook
    mod.set_axon_ntff_profile_hook = lambda h: None
    sys.modules["antenv.axon_hooks"] = mod


def _get_nc():
    if "nc" not in _CACHE:
        os.environ.setdefault("JAX_COMPILATION_CACHE_DIR", "/tmp/jax_comp_cache")
        try:
            import jax
            jax.config.update("jax_compilation_cache_dir", "/tmp/jax_comp_cache")
            jax.config.update("jax_persistent_cache_min_entry_size_bytes", -1)
            jax.config.update("jax_persistent_cache_min_compile_time_secs", 0)
        except Exception:
            pass
        _install_axon_hooks_shim()
        _CACHE["nc"] = _build_nc()
    return _CACHE["nc"]


def _host_thr(tiles):
    """[128, NSLOT] per-slot causal thresholds: mask where (f-p) > thr[s]."""
    thr = np.empty((P, NSLOT), np.float32)
    for s in range(NSLOT):
        thr[:, s] = P * tiles[s] - SLOT_KLEN[s] + 256
    return thr


def make_in_maps(query, key, value):
    query = np.asarray(query, np.float32)
    key = np.asarray(key, np.float32)
    value = np.asarray(value, np.float32)
    in_maps = []
    for core in range(NCORES):
        b, j = divmod(core, 2)
        tiles = TILES_J0 if j == 0 else TILES_J1
        qrows = np.concatenate([query[b, P * t:P * (t + 1)] for t in tiles])
        # qt[s, p, c, q] = qrows[s*128+q, c*128+p]
        qt = np.ascontiguousarray(
            qrows.astype(ml_dtypes.bfloat16)
            .reshape(NSLOT, P, NDC, P).transpose(0, 3, 2, 1))
        # kt[blk, p, c, k] = key[b, blk*512+k, c*128+p]
        kt = np.ascontiguousarray(
            key[b].astype(ml_dtypes.bfloat16)
            .reshape(SK // KTILE, KTILE, NDC, P).transpose(0, 3, 2, 1))
        # v[blk, p, kc, d] = value[b, blk*512 + kc*128 + p, d]
        v = np.ascontiguousarray(
            value[b].astype(ml_dtypes.bfloat16)
            .reshape(SK // KTILE, 4, P, D).transpose(0, 2, 1, 3))
        in_maps.append({
            "qt": qt,
            "kt": kt,
            "v": v,
            "thr": _host_thr(tiles),
        })
    return in_maps


def assemble(results):
    out = np.empty((B, SQ, D), np.float32)
    for core in range(NCORES):
        b, j = divmod(core, 2)
        tiles = TILES_J0 if j == 0 else TILES_J1
        o = results[core]["out"]
        for s, t in enumerate(tiles):
            out[b, P * t:P * (t + 1)] = o[P * s:P * (s + 1)]
    return out


def _get_runner(nc):
    """Build once: a jitted SPMD executable over the 8 axon devices
    (mirrors bass2jax.run_bass_via_pjrt, but cached across kernel() calls
    so repeat calls skip tracing/compilation)."""
    if "runner" in _CACHE:
        return _CACHE["runner"]
    import jax
    import concourse.mybir as mybir
    from concourse import bass2jax
    from jax.sharding import Mesh, PartitionSpec
    from jax.experimental.shard_map import shard_map
    import numpy as _np

    bass2jax.install_neuronx_cc_hook()
    partition_name = (nc.partition_id_tensor.name
                      if nc.partition_id_tensor else None)
    in_names, out_names, out_avals, zero_outs = [], [], [], []
    for alloc in nc.m.functions[0].allocations:
        if not isinstance(alloc, mybir.MemoryLocationSet):
            continue
        name = alloc.memorylocations[0].name
        if alloc.kind == "ExternalInput":
            if name != partition_name:
                in_names.append(name)
        elif alloc.kind == "ExternalOutput":
            out_names.append(name)
            shape = tuple(alloc.tensor_shape)
            dtype = mybir.dt.np(alloc.dtype)
            out_avals.append(jax.core.ShapedArray(shape, dtype))
            zero_outs.append(_np.zeros(shape, dtype))
    n_params = len(in_names)
    all_names = in_names + out_names
    if partition_name is not None:
        all_names = all_names + [partition_name]

    def _body(*args):
        operands = list(args)
        if partition_name is not None:
            operands.append(bass2jax.partition_id_tensor())
        outs = bass2jax._bass_exec_p.bind(
            *operands,
            out_avals=tuple(out_avals),
            in_names=tuple(all_names),
            out_names=tuple(out_names),
            lowering_input_output_aliases=(),
            sim_require_finite=True,
            sim_require_nnan=True,
            nc=nc,
        )
        return tuple(outs)

    devices = jax.devices()[:NCORES]
    mesh = Mesh(_np.asarray(devices), ("core",))
    n_outs = len(out_names)
    sharded = jax.jit(
        shard_map(_body, mesh=mesh,
                  in_specs=(PartitionSpec("core"),) * (n_params + n_outs),
                  out_specs=(PartitionSpec("core"),) * n_outs,
                  check_rep=False),
        donate_argnums=tuple(range(n_params, n_params + n_outs)),
        keep_unused=True,
    )
    _CACHE["runner"] = (sharded, in_names, out_names, out_avals, zero_outs)
    return _CACHE["runner"]


def kernel(query, key, value, _run_kwargs=None):
    import numpy as _np
    nc = _get_nc()
    in_maps = make_in_maps(query, key, value)
    if _run_kwargs is not None:
        # profiling path for test.py
        from concourse.bass_utils import run_bass_kernel_spmd
        res = run_bass_kernel_spmd(nc, in_maps, list(range(NCORES)),
                                   **dict(_run_kwargs))
        _CACHE["last_result"] = res
        return assemble(res.results)
    sharded, in_names, out_names, out_avals, zero_outs = _get_runner(nc)
    concat_in = [
        _np.concatenate([m[name] for m in in_maps], axis=0)
        for name in in_names
    ]
    concat_zeros = [
        _np.zeros((NCORES * z.shape[0], *z.shape[1:]), z.dtype)
        for z in zero_outs
    ]
    out_arrs = sharded(*concat_in, *concat_zeros)
    results = [
        {name: _np.asarray(out_arrs[i]).reshape(NCORES, *out_avals[i].shape)[c]
         for i, name in enumerate(out_names)}
        for c in range(NCORES)
    ]
    return assemble(results)
